# revision 1
# baseline (speedup 1.0000x reference)
"""Class-weighted BCE-with-logits loss on 8 TRN2 NeuronCores.

Math: with sp = softplus(s) and g in {0,1} (so g*g == g):
    l = max(s,0) - s*g + log1p(exp(-|s|)) = sp - s*g
    w = class_weights[g] = cw0 + (cw1-cw0)*g
    sum(l*w) = cw0*T1 + (cw1-cw0)*T2 - cw1*T3
  where T1 = sum(sp), T2 = sum(g*sp), T3 = sum(g*s).

Device work per core (pure data parallel over the batch dim; raw Bass with
explicit semaphores — this walrus build only allows ONE embedded wait per
instruction, so Tile-generated code cannot compile and all waits are
standalone wait_ge instructions):
  ACT:  e = exp(s); sp = ln(e + 1) in place (one table set:
        natural_log_exp_and_others; s ~ N(0,1) so exp(s) cannot overflow
        f32). Ln's accum_out yields per-tile T1 partials for free.
  DVE:  p2 = g*s, p1 = g*sp  (tensor_tensor, bf16, 2x mode)
  PE:   ones[128,1]^T @ p -> [1,512] PSUM accumulated across all tiles,
        giving T2/T3 partials with no extra DVE passes.
Host: shard rows across cores, cast to bf16 (g's {0,1} is exact; s's
rounding is statistically negligible across 33M elements), and do the tiny
final combine of the partials with the class weights in float64.

The scalar engine is the bottleneck (2 LUT passes over every element);
tile sizes ramp small -> large -> small to minimize pipeline fill/drain on
its critical path, and the middle tiles span two 128-row groups (8192 wide)
to amortize per-op overhead.
"""

import os

import numpy as np

B, D = 8192, 4096
N_CORES = 8
SH = B // N_CORES  # rows per core (1024)
P = 128  # SBUF partitions
A = SH // P  # row groups per core (8)
W2 = D  # max tile width
MN = 512  # matmul free-dim chunk (one PSUM bank)

# ("n", group, col_start, width) single-group tile, or ("m", x) merged tile
# covering row groups 2x and 2x+1 at full width. Small tiles first (cut
# ramp-in latency on the scalar engine) and last (short drain).
TILES = (
    [("n", 0, 0, 512), ("n", 0, 512, 1024), ("n", 0, 1536, 2560)]
    + [("n", a, 0, 4096) for a in range(1, A - 1)]
    + [("n", A - 1, 0, 2560), ("n", A - 1, 2560, 1024), ("n", A - 1, 3584, 512)]
)
NT = len(TILES)


def _width(tile):
    return W2 if tile[0] == "m" else tile[3]


S_DTYPE = os.environ.get("K_S_DT", "bfloat16")
G_DTYPE = os.environ.get("K_G_DT", "bfloat16")
E_DTYPE = os.environ.get("K_E_DT", "bfloat16")  # exp(s); also holds sp in place
PROD_DTYPE = "bfloat16"  # products fed to the PE reduction

NBUF = 4  # input stream buffers
KBUF = 2  # intermediate and product buffers

LAST_EXEC_NS = None  # set when _trace=True
LAST_RES = None


def _np_dt(name):
    if name == "bfloat16":
        import ml_dtypes

        return np.dtype(ml_dtypes.bfloat16)
    return np.dtype(name)


def _build():
    import contextlib

    import concourse.bass as bass
    import concourse.mybir as mybir

    mdt = {"float32": mybir.dt.float32, "bfloat16": mybir.dt.bfloat16}
    s_dt = mdt[S_DTYPE]
    g_dt = mdt[G_DTYPE]
    e_dt = mdt[E_DTYPE]
    pr_dt = mdt[PROD_DTYPE]
    f32 = mybir.dt.float32
    AF = mybir.ActivationFunctionType

    nc = bass.Bass()
    s_in = nc.declare_dram_parameter("s", [SH, D], s_dt, isOutput=False)
    g_in = nc.declare_dram_parameter("g", [SH, D], g_dt, isOutput=False)
    t1_out = nc.declare_dram_parameter("t1", [P, NT], f32, isOutput=True)
    t23_out = nc.declare_dram_parameter("t23", [1, 2 * MN], f32, isOutput=True)

    # single-group view [a, p, d] and merged two-group view [x, p, y, d]
    sv_n = s_in.rearrange("(a p) d -> a p d", p=P)
    gv_n = g_in.rearrange("(a p) d -> a p d", p=P)
    sv_m = s_in.rearrange("(x y p) d -> x p y d", y=2, p=P)
    gv_m = g_in.rearrange("(x y p) d -> x p y d", y=2, p=P)

    def dram_aps(t):
        if TILES[t][0] == "m":
            x = TILES[t][1]
            return sv_m[x], gv_m[x]
        _, a, c0, w = TILES[t]
        return sv_n[a][:, c0 : c0 + w], gv_n[a][:, c0 : c0 + w]

    with contextlib.ExitStack() as ctx:
        en = ctx.enter_context
        s_buf = [en(nc.sbuf_tensor(f"s_buf{i}", [P, W2], s_dt)) for i in range(NBUF)]
        g_buf = [en(nc.sbuf_tensor(f"g_buf{i}", [P, W2], g_dt)) for i in range(NBUF)]
        e_buf = [en(nc.sbuf_tensor(f"e_buf{i}", [P, W2], e_dt)) for i in range(KBUF)]
        p1_buf = [en(nc.sbuf_tensor(f"p1_buf{i}", [P, W2], pr_dt)) for i in range(KBUF)]
        p2_buf = [en(nc.sbuf_tensor(f"p2_buf{i}", [P, W2], pr_dt)) for i in range(KBUF)]
        t1_acc = en(nc.sbuf_tensor("t1_acc", [P, NT], f32))
        t23_sb = en(nc.sbuf_tensor("t23_sb", [1, 2 * MN], f32))
        ones = en(nc.sbuf_tensor("ones", [P, 1], pr_dt))
        warm = en(nc.sbuf_tensor("warm", [1, 1], f32))
        scratch = en(nc.sbuf_tensor("scratch", [1, 1], f32))
        flush = en(nc.sbuf_tensor("flush", [1, 128], f32))
        can_s = en(nc.sbuf_tensor("can_s", [P, 2], s_dt))
        can_g = en(nc.sbuf_tensor("can_g", [P, 2], g_dt))
        can_o = en(nc.sbuf_tensor("can_o", [1, 8], f32))
        ps2 = en(nc.psum_tensor("ps2", [1, MN], f32))
        ps3 = en(nc.psum_tensor("ps3", [1, MN], f32))

        s_sem = en(nc.semaphore("s_sem"))
        g_sem = en(nc.semaphore("g_sem"))
        act_sem = en(nc.semaphore("act_sem"))
        dve_sem = en(nc.semaphore("dve_sem"))
        pe_sem = en(nc.semaphore("pe_sem"))
        const_sem = en(nc.semaphore("const_sem"))
        out_sem = en(nc.semaphore("out_sem"))
        block = en(nc.Block(no_gpsimd_drain=True))

        def tail_ap(ap):
            if len(ap.shape) == 3:
                return ap[:, 1, ap.shape[2] - 2 :]
            return ap[:, ap.shape[1] - 2 :]

        def buf_ap(buf, j, t):
            w = _width(TILES[t])
            ap = buf[j][:, 0:w]
            if TILES[t][0] == "m":
                ap = ap.rearrange("p (y d) -> p y d", y=2)
            return ap

        @block.sync
        def _(sync):
            # s-tiles are issued GLAG tiles ahead of g-tiles: ACT (the
            # critical engine) only needs s, and DVE consumes g later.
            GLAG = 3
            for t in range(NT + GLAG):
                if t >= GLAG:
                    u = t - GLAG
                    j = u % NBUF
                    if u >= NBUF:
                        # g slot j was consumed by DVE at tile u-NBUF
                        sync.wait_ge(dve_sem, 2 * (u - NBUF) + 2)
                    _, g_ap = dram_aps(u)
                    sync.dma_start(out=buf_ap(g_buf, j, u), in_=g_ap).then_inc(
                        g_sem, 16
                    )
                    # canary: drains after the parent on the same FIFO ring,
                    # so its completion implies the parent fully landed
                    sync.dma_start(out=can_g[:, :], in_=tail_ap(g_ap)).then_inc(
                        g_sem, 16
                    )
                if t < NT:
                    j = t % NBUF
                    if t >= NBUF:
                        # s slot j consumers: ACT exp and DVE TTs of tile t-NBUF
                        sync.wait_ge(act_sem, t - NBUF + 1)
                        sync.wait_ge(dve_sem, 2 * (t - NBUF) + 2)
                    s_ap, _ = dram_aps(t)
                    sync.dma_start(out=buf_ap(s_buf, j, t), in_=s_ap).then_inc(
                        s_sem, 16
                    )
                    sync.dma_start(out=can_s[:, :], in_=tail_ap(s_ap)).then_inc(
                        s_sem, 16
                    )
            # final outputs
            sync.wait_ge(act_sem, NT + 1)
            sync.dma_start(out=t1_out[:, :], in_=t1_acc[:, :]).then_inc(out_sem, 16)
            sync.wait_ge(dve_sem, 2 * NT + 2)
            sync.dma_start(out=t23_out[:, :], in_=t23_sb[:, :]).then_inc(out_sem, 16)
            # read-back canaries: a DRAM read behind the writes on the same
            # ring implies the output writes drained before the NEFF ends
            sync.dma_start(out=can_o[0:1, 0:4], in_=t1_out[0:1, NT - 4 : NT]).then_inc(
                out_sem, 16
            )
            sync.dma_start(
                out=can_o[0:1, 4:8], in_=t23_out[0:1, 2 * MN - 4 : 2 * MN]
            ).then_inc(out_sem, 16)
            sync.wait_ge(out_sem, 64)

        @block.scalar
        def _(scalar):
            # Dummy ops: (1) walrus places the exp/ln ACT_TABLE_LOAD here so
            # it overlaps the first DMA wait; (2) the accum_out read drains
            # any activation-accumulator residue left by a previous NEFF.
            scalar.memzero(warm[:, :])
            scalar.activation(out=warm[:, :], in_=warm[:, :], func=AF.Exp)
            scalar.activation(
                out=warm[:, :],
                in_=warm[:, :],
                func=AF.Ln,
                bias=1.0,
                accum_out=scratch[:, :],
            )
            for t in range(NT):
                j = t % NBUF
                k = t % KBUF
                w = _width(TILES[t])
                scalar.wait_ge(s_sem, 32 * (t + 1))
                if t >= KBUF:
                    # e/sp slot k is read by DVE's second TT of tile t-KBUF
                    scalar.wait_ge(dve_sem, 2 * (t - KBUF) + 2)
                scalar.activation(
                    out=e_buf[k][:, 0:w], in_=s_buf[j][:, 0:w], func=AF.Exp
                )
                scalar.activation(
                    out=e_buf[k][:, 0:w],
                    in_=e_buf[k][:, 0:w],
                    func=AF.Ln,
                    bias=1.0,
                    accum_out=t1_acc[:, t : t + 1],
                ).then_inc(act_sem, 1)
            # trailing dummy: act_sem == NT+1 implies every accumulator
            # readout (a separate walrus-inserted instruction) has retired
            scalar.activation(
                out=warm[:, :],
                in_=warm[:, :],
                func=AF.Ln,
                bias=1.0,
                accum_out=scratch[:, :],
            ).then_inc(act_sem, 1)

        @block.vector
        def _(vector):
            vector.memset(ones[:, :], 1.0).then_inc(const_sem, 1)
            for t in range(NT):
                j = t % NBUF
                k = t % KBUF
                w = _width(TILES[t])
                vector.wait_ge(s_sem, 32 * (t + 1))
                vector.wait_ge(g_sem, 32 * (t + 1))
                if t >= KBUF:
                    # p1/p2 slot k is read by PE at tile t-KBUF
                    vector.wait_ge(pe_sem, t - KBUF + 1)
                vector.tensor_tensor(
                    out=p2_buf[k][:, 0:w],
                    in0=g_buf[j][:, 0:w],
                    in1=s_buf[j][:, 0:w],
                    op=mybir.AluOpType.mult,
                )
                # incs ride tiny follow-up copies: the inter-op DRAIN
                # guarantees the TT's SBUF writes are visible before the
                # consumer sees the semaphore
                vector.tensor_copy(out=flush[:, :], in_=flush[:, :]).then_inc(
                    dve_sem, 1
                )
                vector.wait_ge(act_sem, t + 1)
                vector.tensor_tensor(
                    out=p1_buf[k][:, 0:w],
                    in0=g_buf[j][:, 0:w],
                    in1=e_buf[k][:, 0:w],
                    op=mybir.AluOpType.mult,
                )
                vector.tensor_copy(out=flush[:, :], in_=flush[:, :]).then_inc(
                    dve_sem, 1
                )
            # copy PSUM partials out once PE is fully done
            vector.wait_ge(pe_sem, NT)
            vector.tensor_copy(out=t23_sb[:, 0:MN], in_=ps2[:, :]).then_inc(dve_sem, 1)
            vector.tensor_copy(out=t23_sb[:, MN : 2 * MN], in_=ps3[:, :])
            vector.tensor_copy(out=flush[:, :], in_=flush[:, :]).then_inc(dve_sem, 1)

        @block.tensor
        def _(tensor):
            tensor.wait_ge(const_sem, 1)
            for t in range(NT):
                k = t % KBUF
                w = _width(TILES[t])
                nch = w // MN
                tensor.wait_ge(dve_sem, 2 * t + 1)
                # redundant stationary reload: separates the semaphore release
                # from the first real read of the freshly-written p2 tile
                tensor.ldweights(ones[:, :])
                for c in range(nch):
                    sl = slice(c * MN, (c + 1) * MN)
                    tensor.matmul(
                        ps3[:, :],
                        ones[:, :],
                        p2_buf[k][:, sl],
                        start=(t == 0 and c == 0),
                        stop=(t == NT - 1 and c == nch - 1),
                    )
                tensor.wait_ge(dve_sem, 2 * t + 2)
                tensor.ldweights(ones[:, :])
                for c in range(nch):
                    sl = slice(c * MN, (c + 1) * MN)
                    tensor.matmul(
                        ps2[:, :],
                        ones[:, :],
                        p1_buf[k][:, sl],
                        start=(t == 0 and c == 0),
                        stop=(t == NT - 1 and c == nch - 1),
                    )
                # reload delays the inc that releases the PSUM readers
                # until the accumulation writes have drained
                tensor.ldweights(ones[:, :]).then_inc(pe_sem, 1)

    return nc


def kernel(s, g, class_weights, _trace=False):
    global LAST_EXEC_NS, LAST_RES
    from concourse.bass_utils import run_bass_kernel_spmd

    s = np.asarray(s)
    g = np.asarray(g)
    cw = np.asarray(class_weights, dtype=np.float64)

    s_np = _np_dt(S_DTYPE)
    g_np = _np_dt(G_DTYPE)

    in_maps = []
    for c in range(N_CORES):
        sl = slice(c * SH, (c + 1) * SH)
        in_maps.append(
            {
                "s": np.ascontiguousarray(s[sl]).astype(s_np, copy=False),
                "g": np.ascontiguousarray(g[sl]).astype(g_np, copy=False),
            }
        )

    nc = _build()
    res = run_bass_kernel_spmd(nc, in_maps, list(range(N_CORES)), trace=_trace)
    LAST_EXEC_NS = res.exec_time_ns
    LAST_RES = res

    total = 0.0
    cw0, cw1 = float(cw[0]), float(cw[1])
    dcw = cw1 - cw0
    for c in range(N_CORES):
        t1 = np.asarray(res.results[c]["t1"], dtype=np.float64).sum()
        t23 = np.asarray(res.results[c]["t23"], dtype=np.float64).reshape(2, -1)
        t2 = t23[0].sum()
        t3 = t23[1].sum()
        total += cw0 * t1 + dcw * t2 - cw1 * t3
    return np.float32(total / (B * D))

